# revision 1
# baseline (speedup 1.0000x reference)
"""Trainium2 Bass kernel for the CorefSeq segment-reduce problem.

Computes, for batch b:
  o[b] = concat([mean of emb[b,s] over s where mentions[b,s]==l for l in (2,3,4)])
  out[b] = relu(o[b] @ W1 + b1) @ W2 + b2

Sharding: data-parallel over the batch axis across 8 NeuronCores
(128 batches per core); classifier weights replicated.

Per-core algorithm (memory-bound: streams the 201MB embeddings slice once):
  - mentions are loaded once, turned into three {0,1} masks (b-major),
    transposed on the TensorEngine into s-major layout, and the per-(b,label)
    1/count factors are computed on-chip.
  - per batch b: one 1.5MB DMA loads emb[b] as [128(s%128), 4(s//128), 768(h)],
    then 8 accumulating matmuls (lhsT = mask columns [s,3], moving = emb rows)
    produce the label sums in PSUM [3, 768]; a ScalarE copy scales by 1/count;
    six TensorE transposes + VectorE copies scatter the result into the
    feature-major o^T [2304, b] activation matrix.
  - one batched MLP over all 128 b at the end (feature-major matmuls).
"""

import sys

import numpy as np

if "/opt/trn_rl_repo" not in sys.path:
    sys.path.insert(0, "/opt/trn_rl_repo")

import concourse.bacc as bacc
import concourse.bass as bass
import concourse.mybir as mybir
import concourse.tile as tile
from concourse.bass_utils import run_bass_kernel_spmd
from concourse.masks import make_identity

N_CORES = 8
B, S, H = 1024, 512, 768
SC = S // 128  # s-chunks of 128 (contraction tiles)
HC = H // 128  # h-chunks of 128 (transpose tiles)
NCLS = 3       # labels (2,3,4) and also output classes
F = NCLS * H   # 2304 concat features
FC = F // 128  # 18
J = 512        # hidden dim
JC = J // 128  # 4

# dtype used for the big segment-sum matmuls. float32r streams the moving
# operand at 1 elem/cycle (vs 4 for float32) at N>=256; masks are exact 0/1
# so only the embedding values see float32r rounding.
MM_DT = mybir.dt.float32r
CAST_MODE = "act"  # "dma": cast during SWDGE transfer; "act": ScalarE copy; "none": f32 matmul

_LAST = {}


def _build(nb: int, mm_dt=None, cast_mode=None) -> bass.Bass:
    mm_dt = MM_DT if mm_dt is None else mm_dt
    cast_mode = CAST_MODE if cast_mode is None else cast_mode
    if mm_dt == mybir.dt.float32:
        cast_mode = "none"
    nc = bacc.Bacc(trn_type="TRN2")
    f32 = mybir.dt.float32

    emb = nc.dram_tensor("embeddings", [nb, S, H], f32, kind="ExternalInput")
    # mentions arrive as int64 viewed as int32 pairs (little-endian: the even
    # columns hold the label values) to dodge jax x64 canonicalization.
    ment = nc.dram_tensor("mentions32", [nb, 2 * S], mybir.dt.int32, kind="ExternalInput")
    w1 = nc.dram_tensor("W1", [F, J], f32, kind="ExternalInput")
    b1 = nc.dram_tensor("b1", [J], f32, kind="ExternalInput")
    w2 = nc.dram_tensor("W2", [J, NCLS], f32, kind="ExternalInput")
    b2 = nc.dram_tensor("b2", [NCLS], f32, kind="ExternalInput")
    out = nc.dram_tensor("out", [nb, NCLS], f32, kind="ExternalOutput")

    with tile.TileContext(nc) as tc:
        with (
            tc.tile_pool(name="consts", bufs=1) as consts,
            tc.tile_pool(name="embp", bufs=4) as embp,
            tc.tile_pool(name="osp", bufs=3) as osp,
            tc.tile_pool(name="psmean", bufs=2, space="PSUM") as psmean,
            tc.tile_pool(name="pssmall", bufs=3, space="PSUM") as pssmall,
        ):
            # identity: gpsimd builds it, DVE re-copies it so its last producer
            # is DVE — PE transposes reading ident + DVE-produced data then
            # carry a single semaphore wait (fused-LDW sync budget).
            ident_g = consts.tile([128, 128], f32)
            make_identity(nc, ident_g)
            ident = consts.tile([128, 128], f32)
            nc.vector.tensor_copy(out=ident, in_=ident_g)

            # ---- mention masks + 1/count factors ----
            m2 = consts.tile([128, 2 * S], mybir.dt.int32)
            nc.sync.dma_start(out=m2[:nb], in_=ment[:, :])
            mentF = consts.tile([128, S], f32)
            nc.vector.tensor_copy(
                out=mentF[:nb], in_=m2.rearrange("p (s two) -> p s two", two=2)[:nb, :, 0]
            )
            maskB = consts.tile([128, NCLS, S], f32)
            cnt = consts.tile([128, NCLS], f32)
            invc = consts.tile([128, NCLS], f32)
            for l in range(NCLS):
                nc.vector.tensor_scalar(
                    out=maskB[:nb, l, :], in0=mentF[:nb], scalar1=float(l + 2),
                    scalar2=None, op0=mybir.AluOpType.is_equal,
                )
                nc.vector.reduce_sum(
                    out=cnt[:nb, l : l + 1], in_=maskB[:nb, l, :], axis=mybir.AxisListType.X
                )
            nc.vector.reciprocal(out=invc[:nb], in_=cnt[:nb])

            # invcT[l, b] — per-partition scalars for the PSUM scale step
            ps_ic = pssmall.tile([NCLS, 128], f32, tag="small")
            nc.tensor.transpose(ps_ic[:, :nb], invc[:nb], ident[:nb, :nb])
            invcT = consts.tile([NCLS, 128], f32)
            nc.vector.tensor_copy(out=invcT[:, :nb], in_=ps_ic[:, :nb])

            # masksT[s%128, c, l, b] — matmul weights (s-major)
            mask_dt = f32 if cast_mode == "none" else mm_dt
            masksT = consts.tile([128, SC, NCLS, 128], mask_dt)
            for c in range(SC):
                for l in range(NCLS):
                    ps_m = pssmall.tile([128, 128], f32, tag="small")
                    nc.tensor.transpose(
                        ps_m[:, :nb], maskB[:nb, l, c * 128 : (c + 1) * 128], ident[:nb, :nb]
                    )
                    nc.vector.tensor_copy(out=masksT[:, c, l, :nb], in_=ps_m[:, :nb])

            # ---- classifier weights (feature-major layouts) ----
            w1sb = consts.tile([128, FC, J], f32)
            nc.sync.dma_start(out=w1sb, in_=w1.rearrange("(kc k) j -> k kc j", k=128))
            b1T = consts.tile([128, JC], f32)
            nc.sync.dma_start(out=b1T, in_=b1.rearrange("(jc j) -> j jc", j=128))
            w2sb = consts.tile([128, JC, NCLS], f32)
            nc.sync.dma_start(out=w2sb, in_=w2.rearrange("(jc j) m -> j jc m", j=128))
            b2T = consts.tile([NCLS, 1], f32)
            nc.sync.dma_start(out=b2T, in_=b2.rearrange("(m one) -> m one", one=1))

            # o^T[feature, b] activation matrix for the MLP
            oT = consts.tile([128, NCLS, HC, 128], f32)


            # ---- main loop: stream embeddings, segment-sum via matmul ----
            # 2 batches per dma_start (3MB transfers: better DMA efficiency,
            # half the SWDGE descriptor-generation rounds)
            BB = 2 if nb % 2 == 0 else 1
            for b0 in range(0, nb, BB):
                src = emb[b0 : b0 + BB].rearrange("bb (c p) h -> p bb c h", p=128)
                if cast_mode == "dma":
                    emb_t = embp.tile([128, BB, SC, H], mm_dt)
                    nc.gpsimd.dma_start(out=emb_t, in_=src)
                elif cast_mode == "act":
                    emb_raw = embp.tile([128, BB, SC, H], f32, tag="embraw", bufs=3)
                    nc.sync.dma_start(out=emb_raw, in_=src)
                    emb_t = embp.tile([128, BB, SC, H], mm_dt, tag="embcast", bufs=2)
                    nc.scalar.copy(out=emb_t, in_=emb_raw)
                else:
                    emb_t = embp.tile([128, BB, SC, H], f32)
                    nc.sync.dma_start(out=emb_t, in_=src)
                for bb in range(BB):
                    b = b0 + bb
                    ps_mean = psmean.tile([NCLS, H], f32)
                    for c in range(SC):
                        lhsT = masksT[:, c, :, b]
                        rhs = emb_t[:, bb, c, :]
                        nc.tensor.matmul(
                            ps_mean[:, 0:512], lhsT, rhs[:, 0:512],
                            start=(c == 0), stop=(c == SC - 1),
                        )
                        nc.tensor.matmul(
                            ps_mean[:, 512:H], lhsT, rhs[:, 512:H],
                            start=(c == 0), stop=(c == SC - 1),
                        )
                    # scale sums -> means while evacuating PSUM (DVE, same engine
                    # as the oT scatter copies so the PE transposes wait on one sem)
                    oS = osp.tile([NCLS, H], f32)
                    nc.vector.tensor_scalar_mul(out=oS, in0=ps_mean, scalar1=invcT[:, b : b + 1])
                    # scatter into oT[l*768 + hc*128 + p, b]
                    for hc in range(HC):
                        ps_t = pssmall.tile([128, NCLS], f32, tag="small")
                        nc.tensor.transpose(
                            ps_t, oS[:, hc * 128 : (hc + 1) * 128], ident[:NCLS, :NCLS]
                        )
                        nc.vector.tensor_copy(out=oT[:, :, hc, b], in_=ps_t)

            # ---- MLP over all b at once (feature-major) ----
            hT = consts.tile([128, JC, 128], f32)
            for jc in range(JC):
                ps_h = pssmall.tile([128, 128], f32, tag="small")
                for kc in range(FC):
                    nc.tensor.matmul(
                        ps_h,
                        w1sb[:, kc, jc * 128 : (jc + 1) * 128],
                        oT[:, kc // HC, kc % HC, :],
                        start=(kc == 0), stop=(kc == FC - 1),
                    )
                nc.scalar.activation(
                    out=hT[:, jc, :], in_=ps_h,
                    func=mybir.ActivationFunctionType.Relu,
                    bias=b1T[:, jc : jc + 1], scale=1.0,
                )
            ps_o = pssmall.tile([NCLS, 128], f32, tag="small")
            for jc in range(JC):
                nc.tensor.matmul(
                    ps_o, w2sb[:, jc, :], hT[:, jc, :],
                    start=(jc == 0), stop=(jc == JC - 1),
                )
            outT = consts.tile([NCLS, 128], f32)
            nc.vector.tensor_scalar_add(out=outT, in0=ps_o, scalar1=b2T[:, 0:1])
            ps_ob = pssmall.tile([128, NCLS], f32, tag="small")
            nc.tensor.transpose(ps_ob[:nb], outT[:, :nb], ident[:NCLS, :NCLS])
            outB = consts.tile([128, NCLS], f32)
            nc.vector.tensor_copy(out=outB[:nb], in_=ps_ob[:nb])
            nc.sync.dma_start(out=out[:, :], in_=outB[:nb])

    if not nc.is_finalized():
        nc.finalize()  # Bacc: reg alloc + semaphore-wait splitting
    return nc


def kernel(embeddings, mentions, W1, b1, W2, b2):
    emb = np.asarray(embeddings, dtype=np.float32)
    ment = np.asarray(mentions)
    if ment.dtype != np.int64:
        ment = ment.astype(np.int64)
    ment32 = np.ascontiguousarray(ment).view(np.int32).reshape(B, 2 * S)
    w1 = np.ascontiguousarray(np.asarray(W1, dtype=np.float32))
    b1a = np.ascontiguousarray(np.asarray(b1, dtype=np.float32))
    w2 = np.ascontiguousarray(np.asarray(W2, dtype=np.float32))
    b2a = np.ascontiguousarray(np.asarray(b2, dtype=np.float32))

    nb = B // N_CORES
    nc = _build(nb)
    in_maps = []
    for i in range(N_CORES):
        sl = slice(i * nb, (i + 1) * nb)
        in_maps.append(
            {
                "embeddings": np.ascontiguousarray(emb[sl]),
                "mentions32": np.ascontiguousarray(ment32[sl]),
                "W1": w1, "b1": b1a, "W2": w2, "b2": b2a,
            }
        )
    res = run_bass_kernel_spmd(nc, in_maps, core_ids=list(range(N_CORES)))
    _LAST["exec_time_ns"] = res.exec_time_ns
    _LAST["result"] = res
    return np.concatenate([res.results[i]["out"] for i in range(N_CORES)], axis=0)

